# revision 46
# baseline (speedup 1.0000x reference)
"""Trainium2 Bass kernel for nn_AdditiveAttention (Bahdanau attention).

Reference computation (B=16, Q=128, K=128, D=512, H=512):
    q = queries @ Wq                     [B,Q,H]
    k = keys @ Wk                        [B,K,H]
    scores[b,q,k] = sum_h wv[h] * tanh(q[b,q,h] + k[b,k,h])
    attn = softmax over valid keys (k < valid_lens[b])
    out = attn @ values                  [B,Q,D]

Strategy (8 NeuronCores, SPMD, key-split data parallelism):
  Work per batch is proportional to its valid_len, and softmax over keys
  decomposes into per-key-range partials (no max subtraction is needed:
  |scores| <= sum|wv| is small).  Each batch's valid key range is split
  into contiguous fragments; fragments are packed into 8 cores x S
  uniform "slots" (cells), one fragment per cell, one SPMD program.  A
  cell computes the UNNORMALIZED partial o = exp(scores) @ values and
  z = sum(exp(scores)) over its key range; the host combines
  out[b] = sum_frag(o) / sum_frag(z).  Slot j has fixed key capacity V_j
  (a host-side search minimizes sum V_j); shorter fragments are masked
  with an additive -1e9.

  On-device per slot: project queriesT/keysT transposed ([h=partitions]);
  per key column k: pre[h,q] = q_projT + k_col broadcast-add on DVE in
  2x_1P packed mode (kproj stored as duplicated (k,k) pairs; qproj read
  as adjacent (q,q+1) pairs -> both operands innermost step-1 bf16);
  tanh on ScalarE in big batched instructions; wv reduction on TensorE
  (lhsT = tanh tile [128h,128q], rhs = wv chunk [128h,1] -> one PSUM
  score column per (k, h-chunk), accumulation order pinned); masked exp
  with fused accumulated sum on ScalarE; transpose of the exp matrix on
  TensorE; and the final exp @ values matmul.
  bf16 on PE/DVE with fp32 PSUM accumulation; tanh/exp fp32 internally.
"""

import os
import sys
import types
import math
import bisect
import numpy as np
import ml_dtypes

# ---------------------------------------------------------------------------
# axon NTFF profile hook (lets trace=True / BASS_TRACE=1 work in this image)
# ---------------------------------------------------------------------------
def _install_axon_hooks():
    if "antenv.axon_hooks" in sys.modules:
        return
    try:
        import trn_agent_boot.trn_boot as _tb

        _hooks = types.ModuleType("antenv.axon_hooks")
        _hook = _tb._ntff_profile_via_ctypes("/opt/axon/libaxon_pjrt.so")
        _hooks.get_axon_ntff_profile_hook = lambda: _hook
        _hooks.set_axon_ntff_profile_hook = lambda h: None
        sys.modules["antenv.axon_hooks"] = _hooks
    except Exception:
        pass


_install_axon_hooks()

import concourse.bass as bass
import concourse.bacc as bacc
import concourse.mybir as mybir
import concourse.tile as tile
import concourse.bass_utils as bass_utils
from concourse.bass_utils import run_bass_kernel_spmd
from concourse.masks import make_identity

# Avoid S3 artifact-upload attempts in the trace path.
bass_utils.upload_artifacts = lambda tmpdir: tmpdir

F32 = mybir.dt.float32
BF16 = mybir.dt.bfloat16
BF16_NP = ml_dtypes.bfloat16

B, Q, K, D, H = 16, 128, 128, 512, 512
NCORES = 8
KT = 16  # key-columns per tanh group
NEG = -1e9

_NC_CACHE: dict = {}
LAST_RESULT = None


def _pack(vl, caps):
    """Pack each batch's valid keys as contiguous ranges into cells (one
    range per cell).  Best-fit: smallest cell that fits the remainder,
    else the largest cell.  Returns content[core][slot] = (b, k0, klen)
    (b = -1 for empty cells) or None if infeasible."""
    cells = []
    for j, cap in enumerate(caps):
        for c in range(NCORES):
            cells.append((cap, c, j))
    avail = sorted(cells)
    content = [[(-1, 0, 0)] * len(caps) for _ in range(NCORES)]
    for b in np.argsort(-vl, kind="stable"):
        rem = int(vl[b])
        k0 = 0
        while rem > 0:
            if not avail:
                return None
            caps_list = [x[0] for x in avail]
            i = bisect.bisect_left(caps_list, rem)
            if i < len(avail):
                cap, c, j = avail.pop(i)
                take = rem
            else:
                cap, c, j = avail.pop()
                take = cap
            content[c][j] = (int(b), k0, take)
            k0 += take
            rem -= take
    return content


def _plan(valid_lens):
    """Search slot capacities minimizing padded work; returns
    (slots, content) with slots = tuple of V_j."""
    vl = np.asarray(valid_lens)
    cand = set()
    for v in vl:
        for k in (1, 2, 3, 4):
            cand.add(int(math.ceil(int(v) / k)))
    cand = sorted(x for x in cand if x >= 1)
    import itertools

    tot = int(vl.sum())
    best = None
    for S in (2, 3, 4):
        for caps in itertools.combinations_with_replacement(
            sorted(cand, reverse=True), S
        ):
            sv = sum(caps)
            if NCORES * sv < tot:
                continue
            if best is not None and Q * sv + S * 700.0 >= best[0]:
                continue
            content = _pack(vl, caps)
            if content is None:
                continue
            best = (Q * sv + S * 700.0, caps, content)
    caps, content = best[1], best[2]
    # Process the smallest slot first: its tiny first tanh fills the
    # ScalarE conveyor early while the big slots' inputs still stream in.
    order = sorted(range(len(caps)), key=lambda j: caps[j])
    order = [order[0]] + sorted(order[1:], key=lambda j: -caps[j])
    caps = tuple(caps[j] for j in order)
    content = [[row[j] for j in order] for row in content]
    return caps, content


def _build_nc(caps):
    """Build + finalize the single-core SPMD program for slot caps."""
    S = len(caps)
    nc = bacc.Bacc(None, target_bir_lowering=False, debug=False)

    qkT = nc.declare_dram_parameter("qkT", [S, 2, D, Q], BF16, isOutput=False)
    vals = nc.declare_dram_parameter("vals", [S, K, D], BF16, isOutput=False)
    wqk_d = nc.declare_dram_parameter("wqk", [2, D, H], BF16, isOutput=False)
    wv_d = nc.declare_dram_parameter("wv4", [128, 4], BF16, isOutput=False)
    mask_d = nc.declare_dram_parameter("mask", [S, 128, K], F32, isOutput=False)
    out_d = nc.declare_dram_parameter("out", [S, Q, D + 1], F32, isOutput=True)

    Tanh = mybir.ActivationFunctionType.Tanh
    Exp = mybir.ActivationFunctionType.Exp

    with tile.TileContext(nc) as tc:
        with (
            tc.tile_pool(name="const", bufs=1) as constp,
            tc.tile_pool(name="io", bufs=1) as iop,
            tc.tile_pool(name="proj", bufs=1) as projp,
            tc.tile_pool(name="stage", bufs=4) as stagep,
            tc.tile_pool(name="sm", bufs=2) as smp,
            tc.tile_pool(name="ps_proj", bufs=3, space="PSUM") as ps_proj,
            tc.tile_pool(name="ps_sc", bufs=3, space="PSUM") as ps_sc,
            tc.tile_pool(name="ps_misc", bufs=1, space="PSUM") as ps_misc,
        ):
            # ---- constants & inputs (critical-path DMAs first) ----------
            wqk_sb = constp.tile([128, 2, 4, H], BF16, tag="wqk")
            wqk_r = wqk_d[:].rearrange("w (c p) h -> p w c h", p=128)
            qkt_sb = iop.tile([128, S, 2, 4, Q], BF16, tag="qkt")
            qkT_r = qkT[:].rearrange("s w (c p) x -> p s w c x", p=128)
            nc.sync.dma_start(wqk_sb[:], wqk_r[:])
            for s in range(S):
                nc.sync.dma_start(qkt_sb[:, s], qkT_r[:, s])
            wq_sb = wqk_sb[:, 0]
            wk_sb = wqk_sb[:, 1]
            qt_sb = qkt_sb[:, :, 0]
            kt_sb = qkt_sb[:, :, 1]
            wv_sb = constp.tile([128, 4], BF16, tag="wv")
            nc.sync.dma_start(wv_sb[:], wv_d[:])
            ident = constp.tile([128, 128], BF16, tag="ident")
            make_identity(nc, ident[:])
            vals_sb = iop.tile([128, S, D], BF16, tag="vals")
            nc.sync.dma_start(vals_sb[:], vals[:].rearrange("s k d -> k s d"))
            mask_sb = iop.tile([128, S, K], F32, tag="mask")
            nc.sync.dma_start(mask_sb[:], mask_d[:].rearrange("s p k -> p s k"))

            # ---- projections: projT[h,x] = sum_d W[d,h] * xT[d,x] -------
            # kproj2 holds each projected key DUPLICATED ([..., k, 2]) so
            # the broadcast-add runs in DVE 2x_1P packed mode: in0 reads
            # the duplicated key pair, in1 adjacent query pairs, keeping
            # pre/tanh tiles contiguous per key column.
            qproj = projp.tile([128, S, 4, Q], BF16, tag="qproj")
            kproj2 = projp.tile([128, S, 4, K, 2], BF16, tag="kproj")

            def project(s):
                V = caps[s]
                for hc in range(4):
                    pq = ps_proj.tile([128, 128], F32, tag="pp", name=f"pq{s}_{hc}")
                    for dc in range(4):
                        nc.tensor.matmul(
                            pq[:],
                            wq_sb[:, dc, hc * 128 : (hc + 1) * 128],
                            qt_sb[:, s, dc, :],
                            start=(dc == 0),
                            stop=(dc == 3),
                        )
                    nc.vector.tensor_copy(qproj[:, s, hc, :], pq[:])
                    pk = ps_proj.tile([128, 128], F32, tag="pp", name=f"pk{s}_{hc}")
                    for dc in range(4):
                        nc.tensor.matmul(
                            pk[:, :V],
                            wk_sb[:, dc, hc * 128 : (hc + 1) * 128],
                            kt_sb[:, s, dc, :V],
                            start=(dc == 0),
                            stop=(dc == 3),
                        )
                    nc.vector.tensor_copy(
                        kproj2[:, s, hc, :V, :],
                        pk[:, :V].unsqueeze(2).broadcast_to((128, V, 2)),
                    )

            # persistent softmax state (cols >= V are never read into live
            # results: the output matmul contracts over eT[:V] only)
            e_sb = projp.tile([128, S, K], BF16, tag="e")

            # ---- main loop ----------------------------------------------
            # Slot epilogues are emitted one slot late: engines are
            # in-order, so emitting an epilogue (which waits on the slot's
            # full PE matmul tail) before the next slot's group work would
            # head-of-line-block every engine at the slot boundary.
            def epilogue(s, psc):
                V = caps[s]
                msc = smp.tile([128, K], F32, tag="msc", name=f"msc{s}")
                nc.vector.tensor_add(
                    msc[:, :V], psc[:, :V], mask_sb[:, s, :V]
                )
                o_sb = smp.tile([128, D + 1], F32, tag="o", name=f"o{s}")
                nc.scalar.activation(
                    e_sb[:, s, :V], msc[:, :V], Exp,
                    accum_out=o_sb[:, D : D + 1],
                )
                pt = ps_misc.tile([128, 128], BF16, tag="pt", name=f"pt{s}")
                nc.tensor.transpose(pt[:], e_sb[:, s, :], ident[:])
                eT = smp.tile([128, 128], BF16, tag="eT", name=f"eT{s}")
                nc.vector.tensor_copy(eT[:], pt[:])
                po = ps_misc.tile([128, D], F32, tag="po", name=f"po{s}")
                nc.tensor.matmul(
                    po[:, :], eT[:V, :], vals_sb[:V, s, :], start=True, stop=True
                )
                nc.vector.tensor_copy(o_sb[:, :D], po[:])
                nc.sync.dma_start(out_d[s], o_sb[:])

            pending = None
            project(0)
            for s in range(S):
                V = caps[s]
                # group sizes: a small leading group lets the first tanh
                # start early (fills the ScalarE conveyor sooner)
                gsizes = []
                rem = V
                if s == 0 and V > 8:
                    gsizes.append(4)
                    rem -= 4
                while rem > 0:
                    gsizes.append(min(KT, rem))
                    rem -= gsizes[-1]
                psc = ps_sc.tile([128, K], F32, tag="psc", name=f"psc{s}")
                prev_last = None
                k0 = 0
                for g, Kg in enumerate(gsizes):
                    nflat = Kg * Q
                    pre = stagep.tile([128, 4, KT * Q], BF16, tag="pre")
                    tnh = stagep.tile([128, 4, KT * Q], BF16, tag="tnh")
                    for hc in range(4):
                        # pre[h, kl, qp, j] = kproj[h, k0+kl] + qproj[h, 2qp+j]
                        in0 = (
                            kproj2[:, s, hc, k0 : k0 + Kg, :]
                            .unsqueeze(2)
                            .broadcast_to((128, Kg, Q // 2, 2))
                        )
                        in1 = (
                            qproj[:, s, hc, :]
                            .rearrange("p (qp j) -> p qp j", j=2)
                            .unsqueeze(1)
                            .broadcast_to((128, Kg, Q // 2, 2))
                        )
                        out = pre[:, hc, :nflat].rearrange(
                            "p (kl qp j) -> p kl qp j", qp=Q // 2, j=2
                        )
                        nc.vector.tensor_add(out, in0, in1)
                    nc.scalar.activation(
                        tnh[:, :, :nflat], pre[:, :, :nflat], Tanh
                    )
                    tnh3 = tnh[:, :, :nflat].rearrange(
                        "p hc (kl q) -> p hc kl q", q=Q
                    )
                    for kl in range(Kg):
                        first = None
                        for hc in range(4):
                            bi = nc.tensor.matmul(
                                psc[:, k0 + kl : k0 + kl + 1],
                                tnh3[:, hc, kl, :],
                                wv_sb[:, hc : hc + 1],
                                start=(hc == 0),
                                stop=(hc == 3),
                            )
                            if hc == 0:
                                first = bi.ins
                            last = bi.ins
                        if prev_last is not None:
                            tile.add_dep_helper(
                                first, prev_last, sync=False,
                                reason="psc accumulation-group order",
                            )
                        prev_last = last
                    k0 += Kg
                    if g == 0 and s + 1 < S:
                        project(s + 1)
                    if g == min(1, len(gsizes) - 1) and pending is not None:
                        epilogue(*pending)
                        pending = None
                if pending is not None:
                    epilogue(*pending)
                pending = (s, psc)
            epilogue(*pending)

    nc.finalize()
    return nc


def kernel(queries, keys, values, valid_lens, Wq, Wk, wv):
    global LAST_RESULT
    queries = np.asarray(queries, dtype=np.float32)
    keys = np.asarray(keys, dtype=np.float32)
    values = np.asarray(values, dtype=np.float32)
    valid_lens = np.asarray(valid_lens, dtype=np.int32)
    Wq = np.asarray(Wq, dtype=np.float32)
    Wk = np.asarray(Wk, dtype=np.float32)
    wv = np.asarray(wv, dtype=np.float32)

    caps, content = _plan(valid_lens)
    S = len(caps)

    if caps not in _NC_CACHE:
        _NC_CACHE[caps] = _build_nc(caps)
    nc = _NC_CACHE[caps]

    # ---- host-side shard prep -------------------------------------------
    wqk = np.stack([Wq, Wk]).astype(BF16_NP)
    wv4 = np.ascontiguousarray(wv.reshape(4, 128).T).astype(BF16_NP)  # [128,4]
    qTt = {
        b: np.ascontiguousarray(queries[b].T).astype(BF16_NP) for b in range(B)
    }

    in_maps = []
    for c in range(NCORES):
        qkTm = np.zeros((S, 2, D, Q), dtype=BF16_NP)
        valsm = np.zeros((S, K, D), dtype=BF16_NP)
        maskm = np.zeros((S, 128, K), dtype=np.float32)
        for s, (b, k0, klen) in enumerate(content[c]):
            if b < 0:
                maskm[s, :, :] = NEG
                continue
            qkTm[s, 0] = qTt[b]
            qkTm[s, 1, :, :klen] = keys[b, k0 : k0 + klen].T.astype(BF16_NP)
            valsm[s, :klen] = values[b, k0 : k0 + klen].astype(BF16_NP)
            maskm[s, :, klen:] = NEG
        in_maps.append(
            {
                "qkT": qkTm,
                "vals": valsm,
                "wqk": wqk,
                "wv4": wv4,
                "mask": maskm,
            }
        )

    res = run_bass_kernel_spmd(nc, in_maps, list(range(NCORES)))
    LAST_RESULT = res

    O = np.zeros((B, Q, D), dtype=np.float64)
    Z = np.zeros((B, Q, 1), dtype=np.float64)
    for c in range(NCORES):
        oz = np.asarray(res.results[c]["out"], dtype=np.float64)
        for s, (b, k0, klen) in enumerate(content[c]):
            if b < 0:
                continue
            O[b] += oz[s, :, :D]
            Z[b] += oz[s, :, D:]
    return (O / Z).astype(np.float32)
